# revision 1
# baseline (speedup 1.0000x reference)
"""LocalWindowAttention Trainium2 kernel (Bass/Tile), 8-core SPMD.

Problem: x[B=4, S=4096, E=512] -> out[B, S, E]
  qkv = x @ W_qkv + b_qkv ; q,k,v = split(qkv)
  scores = (q @ k.T) / sqrt(E), banded mask |i-j| <= 64, softmax
  out = (attn @ v) @ W_out + b_out

Sharding: 8 cores = (batch b in 0..3) x (seq half h in 0..1). Each core owns
2048 query rows and loads a 64-row halo of x on each side (zero-padded at
sequence boundaries), computing q/k/v locally — no collectives.

Per-core layout strategy:
  - x is transposed on host to xT [E, 2176] so the E-contraction of every
    matmul has E on the partition dim.
  - qT/kT produced as [E, rows] (feature on partitions), v as [rows, E].
  - scores tile per 128-query subtile: [128 q, 256 keys] (key window of a
    128-aligned query tile is exactly 256 keys starting 64 left).
  - softmax without max-subtraction (scores are O(1) by construction);
    band + boundary masks are multiplicative 0/1 applied after exp.
  - attn rows are normalized, then transposed via the PE; attended is
    computed directly in transposed form attT [E, q] so the output
    projection needs no further transpose. v's bias is folded in as a
    per-partition bias on the attT copy (sum of normalized attn rows = 1).
  - all matmuls run in float32r (fp32 with 12 low mantissa bits rounded
    away; full PE speed at moving-dim >= 256, ~1.5e-4 rms rel error).

The full-precision fallback (MM_DT=f32) can be selected with env
LWA_MM_F32=1 before first call (4x PE cost on matmuls).
"""

import os
import sys

sys.path.insert(0, "/opt/trn_rl_repo")

import numpy as np

import concourse.bass as bass  # noqa: F401  (registers types)
import concourse.tile as tile
from concourse import bacc, mybir
from concourse.bass_utils import run_bass_kernel_spmd

F32 = mybir.dt.float32
F32R = mybir.dt.float32r

B, S, E = 4, 4096, 512
WINDOW = 64
HALF = S // 2              # 2048 query rows per core
ROWS = HALF + 2 * WINDOW   # 2176 local rows incl. halo
EC = E // 128              # 4 contraction chunks
NT = HALF // 128           # 16 query subtiles per core
NDT = NT // 2              # 8 double tiles
# row slices for the qkv projection moving dim (all >= 256 for f32r speed)
RSLICES = [(0, 512), (512, 512), (1024, 512), (1536, 384), (1920, 256)]

_NC_CACHE = {}


def _round_fp32r(x: np.ndarray) -> np.ndarray:
    """Round-to-nearest fp32 -> fp32r (11-bit mantissa) as walrus expects."""
    u = x.view(np.uint32)
    r = (u.astype(np.uint64) + 0x800) & 0xFFFFF000
    return np.ascontiguousarray(r.astype(np.uint32).view(np.float32))


def _build(mm_f32: bool):
    MM = F32 if mm_f32 else F32R
    nc = bacc.Bacc("TRN2", target_bir_lowering=False, debug=False, num_devices=8)

    xT_d = nc.dram_tensor("xT", [E, ROWS], MM, kind="ExternalInput")
    wqkv_d = nc.dram_tensor("wqkv", [E, 3 * E], MM, kind="ExternalInput")
    bqkv_d = nc.dram_tensor("bqkv", [128, 12], F32, kind="ExternalInput")
    wout_d = nc.dram_tensor("wout", [E, E], MM, kind="ExternalInput")
    bout_d = nc.dram_tensor("bout", [1, E], MM, kind="ExternalInput")
    ones_d = nc.dram_tensor("ones", [1, 128], MM, kind="ExternalInput")
    mask_d = nc.dram_tensor("masks", [128, 3 * 256], F32, kind="ExternalInput")
    zero_d = nc.dram_tensor("zeros", [128, 256], MM, kind="ExternalInput")
    id_d = nc.dram_tensor("ident", [128, 128], F32, kind="ExternalInput")
    out_d = nc.dram_tensor("out", [HALF, E], F32, kind="ExternalOutput")

    ACT = mybir.ActivationFunctionType
    ALU = mybir.AluOpType

    with tile.TileContext(nc) as tc:
        with (
            tc.tile_pool(name="const", bufs=1) as const,
            tc.tile_pool(name="big", bufs=1) as big,
        ):
            # ---- constants ----
            wq_sb = [const.tile([128, 3 * E], MM, name=f"wq{e}", tag=f"wq{e}")
                     for e in range(EC)]
            wo_sb = [const.tile([128, E], MM, name=f"wo{e}", tag=f"wo{e}")
                     for e in range(EC)]
            bq_sb = const.tile([128, 12], F32, name="bq", tag="bq")
            bo_sb = const.tile([1, E], MM, name="bo", tag="bo")
            ones_sb = const.tile([1, 128], MM, name="ones1", tag="ones1")
            mask_sb = const.tile([128, 3 * 256], F32, name="msk", tag="msk")
            id_sb = const.tile([128, 128], F32, name="idn", tag="idn")
            # W_qkv loads split by purpose (q first so projection can
            # start as soon as the first xT row-slices land)
            for e in range(EC):
                nc.sync.dma_start(out=wq_sb[e][:, 0:E],
                                  in_=wqkv_d[128 * e:128 * (e + 1), 0:E])
            nc.sync.dma_start(out=bq_sb, in_=bqkv_d[:, :])

            # ---- persistent products ----
            qkT = [big.tile([128, ROWS], MM, name=f"qkT{f}", tag=f"qkT{f}")
                   for f in range(8)]          # f 0..3 = qT chunks, 4..7 = kT
            v_sb = [big.tile([128, E], MM, name=f"v{r}", tag=f"v{r}")
                    for r in range(ROWS // 128)]   # 17 natural-layout v chunks

            # ---- phase 1: projections (xT pool scoped so attention reuses
            #      its SBUF zone) ----
            with (
                tc.tile_pool(name="xTp", bufs=1) as xTp,
                tc.tile_pool(name="pp", bufs=4, space="PSUM") as pp,
            ):
                xT = [xTp.tile([128, ROWS], MM, name=f"xT{e}", tag=f"xT{e}")
                      for e in range(EC)]
                for si, (r0, ns) in enumerate(RSLICES):
                    for e in range(EC):
                        nc.sync.dma_start(
                            out=xT[e][:, r0:r0 + ns],
                            in_=xT_d[128 * e:128 * (e + 1), r0:r0 + ns])
                    if si == 0:  # k-projection weights after first slice
                        for e in range(EC):
                            nc.sync.dma_start(
                                out=wq_sb[e][:, E:2 * E],
                                in_=wqkv_d[128 * e:128 * (e + 1), E:2 * E])
                # late-phase constants
                for e in range(EC):
                    nc.sync.dma_start(out=wq_sb[e][:, 2 * E:3 * E],
                                      in_=wqkv_d[128 * e:128 * (e + 1), 2 * E:3 * E])
                for e in range(EC):
                    nc.sync.dma_start(out=wo_sb[e],
                                      in_=wout_d[128 * e:128 * (e + 1), :])
                nc.sync.dma_start(out=bo_sb, in_=bout_d[:, :])
                nc.sync.dma_start(out=ones_sb, in_=ones_d[:, :])
                nc.sync.dma_start(out=mask_sb, in_=mask_d[:, :])
                nc.sync.dma_start(out=id_sb, in_=id_d[:, :])

                # qT / kT: [feature, rows]
                for f in range(8):
                    for (r0, ns) in RSLICES:
                        ps = pp.tile([128, 512], F32, name=f"pq{f}_{r0}", tag="pp")
                        for e in range(EC):
                            nc.tensor.matmul(
                                ps[:, :ns],
                                wq_sb[e][:, 128 * f:128 * (f + 1)],
                                xT[e][:, r0:r0 + ns],
                                start=(e == 0), stop=(e == EC - 1),
                            )
                        nc.scalar.activation(
                            out=qkT[f][:, r0:r0 + ns], in_=ps[:, :ns],
                            func=ACT.Identity, bias=bq_sb[:, f:f + 1],
                        )

                # v: [rows, feature] (bias folded into attT copy later)
                for r in range(ROWS // 128):
                    ps = pp.tile([128, 512], F32, name=f"pv{r}", tag="pp")
                    for e in range(EC):
                        nc.tensor.matmul(
                            ps[:],
                            xT[e][:, 128 * r:128 * (r + 1)],
                            wq_sb[e][:, 2 * E:3 * E],
                            start=(e == 0), stop=(e == EC - 1),
                        )
                    nc.vector.tensor_copy(v_sb[r][:], ps[:])

            # ---- phase 2: attention + output projection ----
            with (
                tc.tile_pool(name="attn", bufs=2) as attn,
                tc.tile_pool(name="ptp", bufs=1) as ptp,
                tc.tile_pool(name="ps_s", bufs=2, space="PSUM") as ps_s,
                tc.tile_pool(name="ps_t", bufs=2, space="PSUM") as ps_t,
                tc.tile_pool(name="ps_a", bufs=2, space="PSUM") as ps_a,
                tc.tile_pool(name="pp_out", bufs=2, space="PSUM") as pp_out,
            ):
                # pT0 right half / pT2 left half stay zero for the whole
                # kernel (bufs=1, written halves only)
                pT0 = ptp.tile([128, 256], MM, name="pT0", tag="pT0")
                pT2 = ptp.tile([128, 256], MM, name="pT2", tag="pT2")
                nc.sync.dma_start(out=pT0[:], in_=zero_d[:, :])
                nc.sync.dma_start(out=pT2[:], in_=zero_d[:, :])

                for T in range(NDT):
                    pT1 = attn.tile([128, 256], MM, name=f"pT1_{T}", tag="pT1")
                    for s_half in (0, 1):
                        t = 2 * T + s_half
                        # scores [128 q, 256 keys]
                        ps = ps_s.tile([128, 256], F32, name=f"s{t}", tag="ps_s")
                        for e in range(EC):
                            nc.tensor.matmul(
                                ps[:],
                                qkT[e][:, 64 + 128 * t:192 + 128 * t],
                                qkT[4 + e][:, 128 * t:128 * t + 256],
                                start=(e == 0), stop=(e == EC - 1),
                            )
                        # additive band mask (0 / -1e30), exp with fused
                        # row-sum, then normalize into a fresh tile
                        mi = 0 if t == 0 else (2 if t == NT - 1 else 1)
                        sm = attn.tile([128, 256], F32, name=f"sm{t}", tag="sm")
                        nc.vector.tensor_add(
                            sm[:], ps[:], mask_sb[:, 256 * mi:256 * (mi + 1)])
                        pe_t = attn.tile([128, 256], F32, name=f"pe{t}", tag="pe")
                        rs = attn.tile([128, 1], F32, name=f"rs{t}", tag="rs")
                        nc.scalar.activation(out=pe_t[:], in_=sm[:], func=ACT.Exp,
                                             accum_out=rs[:])
                        rd = attn.tile([128, 1], F32, name=f"rd{t}", tag="rd")
                        nc.vector.reciprocal(rd[:], rs[:])
                        p_t = attn.tile([128, 256], F32, name=f"p{t}", tag="p")
                        nc.vector.tensor_scalar_mul(p_t[:], pe_t[:], rd[:])
                        # transpose both halves onto pT tiles
                        for half in (0, 1):
                            pt_ps = ps_t.tile([128, 128], F32,
                                              name=f"tp{t}_{half}", tag="ps_t")
                            nc.tensor.transpose(
                                pt_ps[:], p_t[:, 128 * half:128 * (half + 1)],
                                id_sb[:])
                            if s_half == 0 and half == 0:
                                dst = pT0[:, 0:128]
                            elif s_half == 0 and half == 1:
                                dst = pT1[:, 0:128]
                            elif s_half == 1 and half == 0:
                                dst = pT1[:, 128:256]
                            else:
                                dst = pT2[:, 128:256]
                            nc.vector.tensor_copy(dst, pt_ps[:])

                    # attended, transposed: attT[e', q(256)]
                    pTs = (pT0, pT1, pT2)
                    attT = []
                    for e in range(EC):
                        pa = ps_a.tile([128, 256], F32, name=f"pa{T}_{e}", tag="ps_a")
                        for kc in range(3):
                            nc.tensor.matmul(
                                pa[:],
                                v_sb[2 * T + kc][:, 128 * e:128 * (e + 1)],
                                pTs[kc][:],
                                start=(kc == 0), stop=(kc == 2),
                            )
                        at = attn.tile([128, 256], MM, name=f"attT{T}_{e}",
                                       tag=f"attT{e}")
                        nc.scalar.activation(
                            out=at[:], in_=pa[:],
                            func=ACT.Identity, bias=bq_sb[:, 8 + e:9 + e],
                        )
                        attT.append(at)

                    # output projection per 128-query subtile
                    for s_half in (0, 1):
                        t = 2 * T + s_half
                        po = pp_out.tile([128, 512], F32, name=f"po{t}", tag="pp_out")
                        for e in range(EC):
                            nc.tensor.matmul(
                                po[:],
                                attT[e][:, 128 * s_half:128 * (s_half + 1)],
                                wo_sb[e][:],
                                start=(e == 0), stop=False,
                            )
                        nc.tensor.matmul(
                            po[:], ones_sb[:], bo_sb[:], start=False, stop=True,
                        )
                        ost = attn.tile([128, 512], F32, name=f"ost{t}", tag="ost")
                        nc.vector.tensor_copy(ost[:], po[:])
                        nc.sync.dma_start(
                            out=out_d[128 * t:128 * (t + 1), :], in_=ost[:])
    nc.compile()
    return nc


def _get_nc():
    mm_f32 = bool(int(os.environ.get("LWA_MM_F32", "0")))
    key = ("nc", mm_f32)
    if key not in _NC_CACHE:
        _NC_CACHE[key] = _build(mm_f32)
    return _NC_CACHE[key], mm_f32


def _prep_shared(W_qkv, b_qkv, W_out, b_out, mm_f32):
    rnd = (lambda a: np.ascontiguousarray(a)) if mm_f32 else _round_fp32r
    scale = 1.0 / np.sqrt(np.float32(E))
    w = np.array(W_qkv, dtype=np.float32, copy=True)
    w[:, :E] *= scale
    b = np.array(b_qkv, dtype=np.float32, copy=True)
    b[:E] *= scale
    shared = {
        "wqkv": rnd(w),
        "bqkv": np.ascontiguousarray(b.reshape(12, 128).T),
        "wout": rnd(np.array(W_out, dtype=np.float32)),
        "bout": rnd(np.array(b_out, dtype=np.float32).reshape(1, E)),
        "ones": rnd(np.ones((1, 128), dtype=np.float32)),
        "zeros": np.zeros((128, 256), dtype=np.float32),
        "ident": np.eye(128, dtype=np.float32),
    }
    return shared


def _masks_for(h: int) -> np.ndarray:
    """Additive masks: 0 where attendable, -1e30 outside the band (or past
    the sequence boundary). Columns: [t0 mask | interior mask | t15 mask]."""
    ii = np.arange(128)[:, None]
    jj = np.arange(256)[None, :]
    band = (jj - ii >= 0) & (jj - ii <= 2 * WINDOW)
    m_mid = band
    m_t0 = band & (jj >= 64) if h == 0 else band
    m_t15 = band & (jj < 192) if h == 1 else band
    stacked = np.concatenate([m_t0, m_mid, m_t15], axis=1)
    return np.ascontiguousarray(
        np.where(stacked, np.float32(0.0), np.float32(-1e30)))


def _install_ntff_shim():
    """The agent image's antenv lacks axon_hooks; synthesize it from the
    boot module's ctypes NTFF driver so trace=True can capture HW timing."""
    import types
    if "antenv.axon_hooks" in sys.modules:
        return
    try:
        from trn_agent_boot.trn_boot import _ntff_profile_via_ctypes
        hook = _ntff_profile_via_ctypes("/opt/axon/libaxon_pjrt.so")
    except Exception:
        hook = None
    mod = types.ModuleType("antenv.axon_hooks")
    mod.get_axon_ntff_profile_hook = lambda: hook
    mod.set_axon_ntff_profile_hook = lambda h: None
    sys.modules["antenv.axon_hooks"] = mod
    # avoid S3 artifact upload attempts during local profile processing
    try:
        from concourse import bass_utils as _bu
        _bu.upload_artifacts = lambda tmpdir: tmpdir
    except Exception:
        pass


def kernel(x, W_qkv, b_qkv, W_out, b_out, _trace=False):
    x = np.asarray(x, dtype=np.float32)
    nc, mm_f32 = _get_nc()
    rnd = (lambda a: np.ascontiguousarray(a)) if mm_f32 else _round_fp32r
    shared = _prep_shared(W_qkv, b_qkv, W_out, b_out, mm_f32)
    masks = [_masks_for(0), _masks_for(1)]

    in_maps = []
    for core in range(8):
        b, h = divmod(core, 2)
        lo = h * HALF - WINDOW
        hi = lo + ROWS
        xh = np.zeros((ROWS, E), dtype=np.float32)
        s0, s1 = max(lo, 0), min(hi, S)
        xh[s0 - lo:s1 - lo] = x[b, s0:s1]
        in_maps.append({
            "xT": rnd(np.ascontiguousarray(xh.T)),
            "masks": masks[h],
            **shared,
        })

    kwargs = {}
    if _trace:
        _install_ntff_shim()
        kwargs = dict(trace=True, trace_cores=[0])
    res = run_bass_kernel_spmd(nc, in_maps, core_ids=list(range(8)), **kwargs)

    out = np.empty((B, S, E), dtype=np.float32)
    for core in range(8):
        b, h = divmod(core, 2)
        out[b, h * HALF:(h + 1) * HALF] = res.results[core]["out"]
    if _trace:
        return out, res
    return out



# revision 3
# speedup vs baseline: 1.1654x; 1.1654x over previous
"""LocalWindowAttention Trainium2 kernel (Bass/Tile), 8-core SPMD.

Problem: x[B=4, S=4096, E=512] -> out[B, S, E]
  qkv = x @ W_qkv + b_qkv ; q,k,v = split(qkv)
  scores = (q @ k.T) / sqrt(E), banded mask |i-j| <= 64, softmax
  out = (attn @ v) @ W_out + b_out

Sharding: 8 cores = (batch b in 0..3) x (seq half h in 0..1). Each core owns
2048 query rows and loads a 64-row halo of x on each side (zero-padded at
sequence boundaries), computing q/k/v locally - no collectives.

Key structural choices vs a straightforward port:
  - W_out is folded into the v-projection on the host:
      (attn @ v) @ W_out = attn @ (x @ (W_v @ W_out))
    so the output projection disappears from the kernel entirely. Since
    attention rows sum to 1, the bias (b_v @ W_out + b_out) is added once
    after attention via a replicated-bias DVE op.
  - All matmul operands are bf16 (1 cycle/row at any moving size, FWL
    weight loads, half the DMA bytes). PSUM accumulation stays fp32.
  - Scores are computed TRANSPOSED, [key, query], by using k-chunks as the
    stationary operand: softmax exp output (bf16, SBUF) is then directly
    the stationary operand of the attended matmul -> no PE transposes and
    no PSUM->SBUF transpose copies at all.
  - Row sums for softmax normalization come from a ones-column appended to
    the v tiles (v_aug[:, h, 256] = 1), so the attended matmul produces
    [q, 256 features + rowsum] per half; normalization by 1/rowsum and the
    folded bias are fused in one scalar_tensor_tensor per half.
  - Inputs stream on two HW DMA queues (SP: xT + output, ACT: weights).
"""

import sys

sys.path.insert(0, "/opt/trn_rl_repo")

import numpy as np
import ml_dtypes

import concourse.bass as bass  # noqa: F401  (registers types)
import concourse.tile as tile
from concourse import bacc, mybir
from concourse.bass_utils import run_bass_kernel_spmd

F32 = mybir.dt.float32
BF16 = mybir.dt.bfloat16
BF16_NP = ml_dtypes.bfloat16

B, S, E = 4, 4096, 512
WINDOW = 64
HALF = S // 2              # 2048 query rows per core
ROWS = HALF + 2 * WINDOW   # 2176 local rows incl. halo
NT = HALF // 128           # 16 query subtiles per core
NCH = ROWS // 128          # 17 v chunks

# xT column DMA slices (small first slice -> earliest possible first matmul)
DSLC = [(0, 320), (320, 512), (832, 512), (1344, 512), (1856, 320)]
# qT matmul slices in xT col space (queries live at local rows [64, 2112))
QSLC = [(64, 256), (320, 512), (832, 512), (1344, 512), (1856, 256)]
# kT matmul slices (full local rows)
KSLC = DSLC

_NC_CACHE = {}


def _build():
    nc = bacc.Bacc("TRN2", target_bir_lowering=False, debug=False, num_devices=8)

    xT_d = nc.dram_tensor("xT", [E, ROWS], BF16, kind="ExternalInput")
    wqkv_d = nc.dram_tensor("wqkv", [E, 3 * E], BF16, kind="ExternalInput")
    bqk_d = nc.dram_tensor("bqk", [128, 8], F32, kind="ExternalInput")
    mask_d = nc.dram_tensor("masks", [128, 768], F32, kind="ExternalInput")
    brep_d = nc.dram_tensor("brep", [128, E], F32, kind="ExternalInput")
    out_d = nc.dram_tensor("out", [HALF, E], BF16, kind="ExternalOutput")

    ACT = mybir.ActivationFunctionType
    ALU = mybir.AluOpType

    with tile.TileContext(nc) as tc:
        with (
            tc.tile_pool(name="const", bufs=1) as const,
            tc.tile_pool(name="big", bufs=1) as big,
        ):
            # ---- constants ----
            wq_sb = [const.tile([128, 3 * E], BF16, name=f"wq{e}", tag=f"wq{e}")
                     for e in range(4)]
            bqk_sb = const.tile([128, 8], F32, name="bqk", tag="bqk")
            mask_sb = const.tile([128, 6, 128], F32, name="msk", tag="msk")
            brep_sb = const.tile([128, E], F32, name="brep", tag="brep")

            # ---- persistent products ----
            qT = [big.tile([128, HALF], BF16, name=f"qT{f}", tag=f"qT{f}")
                  for f in range(4)]
            kT = [big.tile([128, ROWS], BF16, name=f"kT{f}", tag=f"kT{f}")
                  for f in range(4)]
            # v rows with W_out folded in; per 128-row chunk: [h, 257] where
            # col 256 of each half is 1.0 (rowsum column for softmax denom)
            vaug = [big.tile([128, 2, 257], BF16, name=f"v{r}", tag=f"v{r}")
                    for r in range(NCH)]

            xTp = [big.tile([128, ROWS], BF16, name=f"xT{e}", tag=f"xT{e}")
                   for e in range(4)]

            # ones columns for the rowsum trick (off critical path, DVE)
            for r in range(NCH):
                nc.vector.memset(vaug[r][:, :, 256:257], 1.0)

            # ---- input DMAs ----
            # SP queue: first xT slice, then the rest slice-major
            # ACT queue: q-weights first, then k, then v(=W_v@W_out), consts
            for e in range(4):
                nc.sync.dma_start(out=xTp[e][:, 0:320],
                                  in_=xT_d[128 * e:128 * (e + 1), 0:320])
            for e in range(4):
                nc.scalar.dma_start(out=wq_sb[e][:, 0:E],
                                    in_=wqkv_d[128 * e:128 * (e + 1), 0:E])
            nc.scalar.dma_start(out=bqk_sb, in_=bqk_d[:, :])
            for (c0, w) in DSLC[1:]:
                for e in range(4):
                    nc.sync.dma_start(out=xTp[e][:, c0:c0 + w],
                                      in_=xT_d[128 * e:128 * (e + 1), c0:c0 + w])
            for e in range(4):
                nc.scalar.dma_start(out=wq_sb[e][:, E:2 * E],
                                    in_=wqkv_d[128 * e:128 * (e + 1), E:2 * E])
            for e in range(4):
                nc.scalar.dma_start(out=wq_sb[e][:, 2 * E:3 * E],
                                    in_=wqkv_d[128 * e:128 * (e + 1), 2 * E:3 * E])
            nc.scalar.dma_start(out=mask_sb[:, :, :], in_=mask_d[:, :])
            nc.scalar.dma_start(out=brep_sb, in_=brep_d[:, :])

            # ---- phase 1: projections ----
            with tc.tile_pool(name="pp", bufs=4, space="PSUM") as pp:
                # qT / kT slice-major so compute starts after the first slice
                for si in range(5):
                    q0, qn = QSLC[si]
                    k0, kn = KSLC[si]
                    for f in range(4):
                        ps = pp.tile([128, 512], F32, name=f"pq{f}_{si}", tag="pp")
                        for e in range(4):
                            nc.tensor.matmul(
                                ps[:, :qn],
                                wq_sb[e][:, 128 * f:128 * (f + 1)],
                                xTp[e][:, q0:q0 + qn],
                                start=(e == 0), stop=(e == 3),
                            )
                        nc.scalar.activation(
                            out=qT[f][:, q0 - 64:q0 - 64 + qn], in_=ps[:, :qn],
                            func=ACT.Identity, bias=bqk_sb[:, f:f + 1],
                        )
                    for f in range(4):
                        ps = pp.tile([128, 512], F32, name=f"pk{f}_{si}", tag="pp")
                        for e in range(4):
                            nc.tensor.matmul(
                                ps[:, :kn],
                                wq_sb[e][:, E + 128 * f:E + 128 * (f + 1)],
                                xTp[e][:, k0:k0 + kn],
                                start=(e == 0), stop=(e == 3),
                            )
                        nc.scalar.activation(
                            out=kT[f][:, k0:k0 + kn], in_=ps[:, :kn],
                            func=ACT.Identity, bias=bqk_sb[:, 4 + f:5 + f],
                        )
                # v' = x @ (W_v @ W_out), natural [rows, feat] layout
                for r in range(NCH):
                    ps = pp.tile([128, 2, 256], F32, name=f"pv{r}", tag="pp")
                    for e in range(4):
                        nc.tensor.matmul(
                            ps[:, :, :],
                            xTp[e][:, 128 * r:128 * (r + 1)],
                            wq_sb[e][:, 2 * E:3 * E],
                            start=(e == 0), stop=(e == 3),
                        )
                    nc.vector.tensor_copy(vaug[r][:, :, 0:256], ps[:, :, :])

            # ---- phase 2: banded attention, output written directly ----
            with (
                tc.tile_pool(name="attn", bufs=2) as attn,
                tc.tile_pool(name="ps_s", bufs=2, space="PSUM") as ps_s,
                tc.tile_pool(name="ps_a", bufs=4, space="PSUM") as ps_a,
            ):
                for t in range(NT):
                    # scores transposed: [key, query], two 128-key chunks
                    ps3 = ps_s.tile([128, 2, 128], F32, name=f"s{t}", tag="ps_s")
                    for kc in range(2):
                        for f in range(4):
                            nc.tensor.matmul(
                                ps3[:, kc:kc + 1, :],
                                kT[f][:, 128 * (t + kc):128 * (t + kc + 1)],
                                qT[f][:, 128 * t:128 * (t + 1)],
                                start=(f == 0), stop=(f == 3),
                            )
                    # additive band mask (0 / -1e30), then exp -> bf16 SBUF
                    mi = 0 if t == 0 else (2 if t == NT - 1 else 1)
                    nc.vector.tensor_add(
                        ps3[:, :, :], ps3[:, :, :],
                        mask_sb[:, 2 * mi:2 * mi + 2, :])
                    ept = attn.tile([128, 2, 128], BF16, name=f"pe{t}", tag="pe")
                    nc.scalar.activation(out=ept[:, :, :], in_=ps3[:, :, :],
                                         func=ACT.Exp)
                    # attended (unnormalized) + rowsum via the ones column
                    paA = ps_a.tile([128, 257], F32, name=f"paA{t}", tag="ps_a")
                    paB = ps_a.tile([128, 257], F32, name=f"paB{t}", tag="ps_a")
                    for kc in range(2):
                        nc.tensor.matmul(
                            paA[:, :], ept[:, kc:kc + 1, :],
                            vaug[t + kc][:, 0:1, :],
                            start=(kc == 0), stop=(kc == 1),
                        )
                        nc.tensor.matmul(
                            paB[:, :], ept[:, kc:kc + 1, :],
                            vaug[t + kc][:, 1:2, :],
                            start=(kc == 0), stop=(kc == 1),
                        )
                    rd = attn.tile([128, 1], F32, name=f"rd{t}", tag="rd")
                    nc.vector.reciprocal(rd[:], paA[:, 256:257])
                    # out = attended * (1/rowsum) + folded bias, cast to bf16
                    ost = attn.tile([128, 512], BF16, name=f"ost{t}", tag="ost")
                    nc.vector.scalar_tensor_tensor(
                        ost[:, 0:256], paA[:, 0:256], rd[:], brep_sb[:, 0:256],
                        op0=ALU.mult, op1=ALU.add,
                    )
                    nc.vector.scalar_tensor_tensor(
                        ost[:, 256:512], paB[:, 0:256], rd[:], brep_sb[:, 256:512],
                        op0=ALU.mult, op1=ALU.add,
                    )
                    nc.sync.dma_start(
                        out=out_d[128 * t:128 * (t + 1), :], in_=ost[:])
    nc.compile()
    return nc


def _get_nc():
    if "nc" not in _NC_CACHE:
        _NC_CACHE["nc"] = _build()
    return _NC_CACHE["nc"]


def _prep_shared(W_qkv, b_qkv, W_out, b_out):
    scale = 1.0 / np.sqrt(np.float64(E))
    W = np.array(W_qkv, dtype=np.float64)
    Wo = np.array(W_out, dtype=np.float64)
    b = np.array(b_qkv, dtype=np.float64)
    bo = np.array(b_out, dtype=np.float64)

    wq = W[:, :E] * scale
    wk = W[:, E:2 * E]
    wvo = W[:, 2 * E:3 * E] @ Wo          # fold output projection into v
    wqkv = np.concatenate([wq, wk, wvo], axis=1)

    bq = b[:E] * scale
    bk = b[E:2 * E]
    bqk = np.stack([*(bq.reshape(4, 128)), *(bk.reshape(4, 128))], axis=1)
    bvo = b[2 * E:3 * E] @ Wo + bo        # folded output bias

    shared = {
        "wqkv": np.ascontiguousarray(wqkv.astype(np.float32)).astype(BF16_NP),
        "bqk": np.ascontiguousarray(bqk.astype(np.float32)),
        "brep": np.ascontiguousarray(
            np.tile(bvo.astype(np.float32)[None, :], (128, 1))),
    }
    return shared


def _masks_for(h: int) -> np.ndarray:
    """Additive masks in TRANSPOSED [key-in-chunk, kc, query] layout.
    Variant blocks along dim1: [t0 (2x128) | interior | t_last]."""
    j = np.arange(128)[:, None, None]     # key index within chunk
    kc = np.arange(2)[None, :, None]
    i = np.arange(128)[None, None, :]     # query index within tile
    jj = 128 * kc + j                     # key position in the 256 window
    band = (jj - i >= 0) & (jj - i <= 2 * WINDOW)
    m_mid = band
    m_t0 = band & (jj >= 64) if h == 0 else band
    m_tl = band & (jj < 192) if h == 1 else band
    stacked = np.concatenate([m_t0, m_mid, m_tl], axis=1)   # [128, 6, 128]
    return np.ascontiguousarray(
        np.where(stacked, np.float32(0.0), np.float32(-1e30)).reshape(128, 768))


def _install_ntff_shim():
    """The agent image's antenv lacks axon_hooks; synthesize it from the
    boot module's ctypes NTFF driver so trace=True can capture HW timing."""
    import types
    if "antenv.axon_hooks" in sys.modules:
        return
    try:
        from trn_agent_boot.trn_boot import _ntff_profile_via_ctypes
        hook = _ntff_profile_via_ctypes("/opt/axon/libaxon_pjrt.so")
    except Exception:
        hook = None
    mod = types.ModuleType("antenv.axon_hooks")
    mod.get_axon_ntff_profile_hook = lambda: hook
    mod.set_axon_ntff_profile_hook = lambda h: None
    sys.modules["antenv.axon_hooks"] = mod
    # avoid S3 artifact upload attempts during local profile processing
    try:
        from concourse import bass_utils as _bu
        _bu.upload_artifacts = lambda tmpdir: tmpdir
    except Exception:
        pass


def kernel(x, W_qkv, b_qkv, W_out, b_out, _trace=False):
    x = np.asarray(x, dtype=np.float32)
    nc = _get_nc()
    shared = _prep_shared(W_qkv, b_qkv, W_out, b_out)
    masks = [_masks_for(0), _masks_for(1)]

    in_maps = []
    for core in range(8):
        b, h = divmod(core, 2)
        lo = h * HALF - WINDOW
        hi = lo + ROWS
        xh = np.zeros((ROWS, E), dtype=np.float32)
        s0, s1 = max(lo, 0), min(hi, S)
        xh[s0 - lo:s1 - lo] = x[b, s0:s1]
        in_maps.append({
            "xT": np.ascontiguousarray(xh.T).astype(BF16_NP),
            "masks": masks[h],
            **shared,
        })

    kwargs = {}
    if _trace:
        _install_ntff_shim()
        kwargs = dict(trace=True, trace_cores=[0])
    res = run_bass_kernel_spmd(nc, in_maps, core_ids=list(range(8)), **kwargs)

    out = np.empty((B, S, E), dtype=np.float32)
    for core in range(8):
        b, h = divmod(core, 2)
        out[b, h * HALF:(h + 1) * HALF] = res.results[core]["out"].astype(np.float32)
    if _trace:
        return out, res
    return out


# revision 6
# speedup vs baseline: 1.2961x; 1.1121x over previous
"""LocalWindowAttention Trainium2 kernel (Bass/Tile), 8-core SPMD.

Problem: x[B=4, S=4096, E=512] -> out[B, S, E]
  qkv = x @ W_qkv + b_qkv ; q,k,v = split(qkv)
  scores = (q @ k.T) / sqrt(E), banded mask |i-j| <= 64, softmax
  out = (attn @ v) @ W_out + b_out

Sharding: 8 cores = (batch b in 0..3) x (seq half h in 0..1). Each core owns
2048 query rows and loads a 64-row halo of x on each side (zero-padded at
sequence boundaries), computing q/k/v locally - no collectives.

Key structural choices:
  - W_out is folded into the v-projection on the host:
      (attn @ v) @ W_out = attn @ (x @ (W_v @ W_out))
    so the output projection disappears from the kernel. Since attention
    rows sum to 1, the output bias (b_v @ W_out + b_out) is folded into
    the v rows themselves (v'' = v' + b_vo added during the PSUM->SBUF
    copy), which makes softmax normalization a pure per-partition scale.
  - All matmul operands are bf16 (1 cycle/row at any moving size, FWL
    weight loads, half the DMA bytes). PSUM accumulation stays fp32.
  - Scores are computed TRANSPOSED, [key, query], with k-chunks as the
    stationary operand: the exp output is directly the stationary operand
    of the attended matmul -> no PE transposes at all.
  - The band mask is MULTIPLICATIVE (0/1 bf16) applied after exp on the
    gpsimd engine (SBUF-only op; raw scores are O(1) so unmasked exp is
    safe), keeping both the DVE and the PSUM out of that step.
  - Row sums for softmax come from a ones-column appended to the v tiles
    (attended matmul emits [q, 256 feats + rowsum] per half); the final
    normalize is scalar-engine activation with per-partition scale 1/rowsum.
  - Inputs stream on two HW DMA queues (SP: xT + output, ACT: weights),
    ordered so the PE starts in ~5us and never starves.
"""

import sys

sys.path.insert(0, "/opt/trn_rl_repo")

import numpy as np
import ml_dtypes

import concourse.bass as bass  # noqa: F401  (registers types)
import concourse.tile as tile
from concourse import bacc, mybir
from concourse.bass_utils import run_bass_kernel_spmd

F32 = mybir.dt.float32
BF16 = mybir.dt.bfloat16
BF16_NP = ml_dtypes.bfloat16

B, S, E = 4, 4096, 512
WINDOW = 64
HALF = S // 2              # 2048 query rows per core
ROWS = HALF + 2 * WINDOW   # 2176 local rows incl. halo
NT = HALF // 128           # 16 query subtiles per core
NCH = ROWS // 128          # 17 v chunks

# xT column DMA slices (small first slice -> earliest possible first matmul)
DSLC = [(0, 256), (256, 512), (768, 512), (1280, 512), (1792, 384)]
# qT matmul groups in xT col space (queries live at local rows [64, 2112))
QSLC = [(64, 192), (256, 512), (768, 512), (1280, 512), (1792, 320)]
# kT matmul groups (full local rows)
KSLC = [(0, 256), (256, 512), (768, 512), (1280, 512), (1792, 384)]

_NC_CACHE = {}


def _build():
    nc = bacc.Bacc("TRN2", target_bir_lowering=False, debug=False, num_devices=8)

    xT_d = nc.dram_tensor("xT", [E, ROWS], BF16, kind="ExternalInput")
    wqkv_d = nc.dram_tensor("wqkv", [E, 3 * E], BF16, kind="ExternalInput")
    bqk_d = nc.dram_tensor("bqk", [128, 8], F32, kind="ExternalInput")
    mask_d = nc.dram_tensor("masks", [128, 768], BF16, kind="ExternalInput")
    brep_d = nc.dram_tensor("brep", [128, E], F32, kind="ExternalInput")
    out_d = nc.dram_tensor("out", [HALF, E], BF16, kind="ExternalOutput")

    ACT = mybir.ActivationFunctionType
    ALU = mybir.AluOpType

    with tile.TileContext(nc) as tc:
        with (
            tc.tile_pool(name="const", bufs=1) as const,
            tc.tile_pool(name="big", bufs=1) as big,
        ):
            # ---- constants ----
            wq_sb = [const.tile([128, 3 * E], BF16, name=f"wq{e}", tag=f"wq{e}")
                     for e in range(4)]
            bqk_sb = const.tile([128, 8], F32, name="bqk", tag="bqk")
            mask_sb = const.tile([128, 6, 128], BF16, name="msk", tag="msk")
            brep_sb = const.tile([128, 2, 256], F32, name="brep", tag="brep")

            # ---- persistent products ----
            qT = [big.tile([128, HALF], BF16, name=f"qT{f}", tag=f"qT{f}")
                  for f in range(4)]
            kT = [big.tile([128, ROWS], BF16, name=f"kT{f}", tag=f"kT{f}")
                  for f in range(4)]
            # v rows with W_out and output bias folded in; per 128-row chunk:
            # [h, 257] where col 256 of each half is 1.0 (rowsum column)
            vaug = [big.tile([128, 2, 257], BF16, name=f"v{r}", tag=f"v{r}")
                    for r in range(NCH)]

            xTp = [big.tile([128, ROWS], BF16, name=f"xT{e}", tag=f"xT{e}")
                   for e in range(4)]

            # ones columns for the rowsum trick (off critical path)
            for r in range(NCH):
                nc.gpsimd.memset(vaug[r][:, :, 256:257], 1.0)

            # ---- input DMAs ----
            # ACT queue: biases, then weight chunks interleaved in the order
            # the projection groups consume them. SP queue: xT slice-major.
            nc.scalar.dma_start(out=bqk_sb, in_=bqk_d[:, :])
            for e in range(4):
                nc.scalar.dma_start(out=wq_sb[e][:, 0:256],
                                    in_=wqkv_d[128 * e:128 * (e + 1), 0:256])
            for e in range(4):
                nc.sync.dma_start(out=xTp[e][:, 0:256],
                                  in_=xT_d[128 * e:128 * (e + 1), 0:256])
            for e in range(4):
                nc.scalar.dma_start(out=wq_sb[e][:, E:E + 256],
                                    in_=wqkv_d[128 * e:128 * (e + 1), E:E + 256])
            for e in range(4):
                nc.scalar.dma_start(out=wq_sb[e][:, 256:512],
                                    in_=wqkv_d[128 * e:128 * (e + 1), 256:512])
            for e in range(4):
                nc.scalar.dma_start(
                    out=wq_sb[e][:, E + 256:2 * E],
                    in_=wqkv_d[128 * e:128 * (e + 1), E + 256:2 * E])
            for (c0, w) in DSLC[1:]:
                for e in range(4):
                    nc.sync.dma_start(out=xTp[e][:, c0:c0 + w],
                                      in_=xT_d[128 * e:128 * (e + 1), c0:c0 + w])
            for e in range(4):
                nc.scalar.dma_start(out=wq_sb[e][:, 2 * E:3 * E],
                                    in_=wqkv_d[128 * e:128 * (e + 1), 2 * E:3 * E])
            nc.scalar.dma_start(out=mask_sb[:, :, :], in_=mask_d[:, :])
            nc.scalar.dma_start(out=brep_sb[:, :, :], in_=brep_d[:, :])

            # ---- phase 1: projections ----
            with tc.tile_pool(name="pp", bufs=4, space="PSUM") as pp:
                # qT / kT slice-major so compute starts after the first slice
                for si in range(5):
                    q0, qn = QSLC[si]
                    for f in range(4):
                        ps = pp.tile([128, 512], F32,
                                     name=f"pq{f}_{si}", tag="pp")
                        for e in range(4):
                            nc.tensor.matmul(
                                ps[:, :qn],
                                wq_sb[e][:, 128 * f:128 * (f + 1)],
                                xTp[e][:, q0:q0 + qn],
                                start=(e == 0), stop=(e == 3),
                            )
                        nc.vector.tensor_scalar_add(
                            qT[f][:, q0 - 64:q0 - 64 + qn], ps[:, :qn],
                            bqk_sb[:, f:f + 1],
                        )
                    k0, kn = KSLC[si]
                    for f in range(4):
                        ps = pp.tile([128, 512], F32,
                                     name=f"pk{f}_{si}", tag="pp")
                        for e in range(4):
                            nc.tensor.matmul(
                                ps[:, :kn],
                                wq_sb[e][:, E + 128 * f:E + 128 * (f + 1)],
                                xTp[e][:, k0:k0 + kn],
                                start=(e == 0), stop=(e == 3),
                            )
                        nc.scalar.activation(
                            out=kT[f][:, k0:k0 + kn], in_=ps[:, :kn],
                            func=ACT.Identity, bias=bqk_sb[:, 4 + f:5 + f],
                        )
                # v'' = x @ (W_v @ W_out) + (b_v @ W_out + b_out), natural
                # [rows, feat] layout (bias add fused into the PSUM copy)
                for r in range(NCH):
                    ps = pp.tile([128, 2, 256], F32, name=f"pv{r}", tag="pp")
                    for e in range(4):
                        nc.tensor.matmul(
                            ps[:, :, :],
                            xTp[e][:, 128 * r:128 * (r + 1)],
                            wq_sb[e][:, 2 * E:3 * E],
                            start=(e == 0), stop=(e == 3),
                        )
                    nc.vector.tensor_add(
                        vaug[r][:, :, 0:256], ps[:, :, :], brep_sb[:, :, :])

            # ---- phase 2: banded attention, output written directly ----
            with (
                tc.tile_pool(name="attn", bufs=3) as attn,
                tc.tile_pool(name="ps_s", bufs=3, space="PSUM") as ps_s,
                tc.tile_pool(name="ps_a", bufs=4, space="PSUM") as ps_a,
            ):
                for t in range(NT):
                    # scores transposed: [key, query], two 128-key chunks
                    ps3 = ps_s.tile([128, 2, 128], F32, name=f"s{t}", tag="ps_s")
                    for kc in range(2):
                        for f in range(4):
                            nc.tensor.matmul(
                                ps3[:, kc:kc + 1, :],
                                kT[f][:, 128 * (t + kc):128 * (t + kc + 1)],
                                qT[f][:, 128 * t:128 * (t + 1)],
                                start=(f == 0), stop=(f == 3),
                            )
                    # exp (raw scores are O(1)), then multiplicative 0/1
                    # band mask on gpsimd (SBUF-only), per key-chunk
                    mi = 0 if t == 0 else (2 if t == NT - 1 else 1)
                    ept = attn.tile([128, 2, 128], BF16, name=f"pe{t}", tag="pe")
                    for kc in range(2):
                        nc.scalar.activation(
                            out=ept[:, kc:kc + 1, :], in_=ps3[:, kc:kc + 1, :],
                            func=ACT.Exp)
                        nc.gpsimd.tensor_tensor(
                            ept[:, kc:kc + 1, :], ept[:, kc:kc + 1, :],
                            mask_sb[:, 2 * mi + kc:2 * mi + kc + 1, :],
                            op=ALU.mult)
                    # attended (unnormalized) + rowsum via the ones column
                    paA = ps_a.tile([128, 257], F32, name=f"paA{t}", tag="ps_a")
                    paB = ps_a.tile([128, 257], F32, name=f"paB{t}", tag="ps_a")
                    for kc in range(2):
                        nc.tensor.matmul(
                            paA[:, :], ept[:, kc:kc + 1, :],
                            vaug[t + kc][:, 0:1, :],
                            start=(kc == 0), stop=(kc == 1),
                        )
                        nc.tensor.matmul(
                            paB[:, :], ept[:, kc:kc + 1, :],
                            vaug[t + kc][:, 1:2, :],
                            start=(kc == 0), stop=(kc == 1),
                        )
                    rd = attn.tile([128, 1], F32, name=f"rd{t}", tag="rd")
                    nc.vector.reciprocal(rd[:], paA[:, 256:257])
                    # out = attended * (1/rowsum); bias already in v rows
                    ost = attn.tile([128, 512], BF16, name=f"ost{t}", tag="ost")
                    nc.scalar.activation(
                        out=ost[:, 0:256], in_=paA[:, 0:256],
                        func=ACT.Identity, scale=rd[:])
                    nc.scalar.activation(
                        out=ost[:, 256:512], in_=paB[:, 0:256],
                        func=ACT.Identity, scale=rd[:])
                    nc.sync.dma_start(
                        out=out_d[128 * t:128 * (t + 1), :], in_=ost[:])
    nc.compile()
    return nc


def _get_nc():
    if "nc" not in _NC_CACHE:
        _NC_CACHE["nc"] = _build()
    return _NC_CACHE["nc"]


def _prep_shared(W_qkv, b_qkv, W_out, b_out):
    scale = 1.0 / np.sqrt(np.float64(E))
    W = np.array(W_qkv, dtype=np.float64)
    Wo = np.array(W_out, dtype=np.float64)
    b = np.array(b_qkv, dtype=np.float64)
    bo = np.array(b_out, dtype=np.float64)

    wq = W[:, :E] * scale
    wk = W[:, E:2 * E]
    wvo = W[:, 2 * E:3 * E] @ Wo          # fold output projection into v
    wqkv = np.concatenate([wq, wk, wvo], axis=1)

    bq = b[:E] * scale
    bk = b[E:2 * E]
    bqk = np.stack([*(bq.reshape(4, 128)), *(bk.reshape(4, 128))], axis=1)
    bvo = b[2 * E:3 * E] @ Wo + bo        # folded output bias

    shared = {
        "wqkv": np.ascontiguousarray(wqkv.astype(np.float32)).astype(BF16_NP),
        "bqk": np.ascontiguousarray(bqk.astype(np.float32)),
        "brep": np.ascontiguousarray(
            np.tile(bvo.astype(np.float32)[None, :], (128, 1))),
    }
    return shared


def _masks_for(h: int) -> np.ndarray:
    """Multiplicative 0/1 masks in TRANSPOSED [key-in-chunk, kc, query]
    layout. Variant blocks along dim1: [t0 (2x128) | interior | t_last]."""
    j = np.arange(128)[:, None, None]     # key index within chunk
    kc = np.arange(2)[None, :, None]
    i = np.arange(128)[None, None, :]     # query index within tile
    jj = 128 * kc + j                     # key position in the 256 window
    band = (jj - i >= 0) & (jj - i <= 2 * WINDOW)
    m_mid = band
    m_t0 = band & (jj >= 64) if h == 0 else band
    m_tl = band & (jj < 192) if h == 1 else band
    stacked = np.concatenate([m_t0, m_mid, m_tl], axis=1)   # [128, 6, 128]
    return np.ascontiguousarray(
        stacked.reshape(128, 768).astype(np.float32)).astype(BF16_NP)


def _install_ntff_shim():
    """The agent image's antenv lacks axon_hooks; synthesize it from the
    boot module's ctypes NTFF driver so trace=True can capture HW timing."""
    import types
    if "antenv.axon_hooks" in sys.modules:
        return
    try:
        from trn_agent_boot.trn_boot import _ntff_profile_via_ctypes
        hook = _ntff_profile_via_ctypes("/opt/axon/libaxon_pjrt.so")
    except Exception:
        hook = None
    mod = types.ModuleType("antenv.axon_hooks")
    mod.get_axon_ntff_profile_hook = lambda: hook
    mod.set_axon_ntff_profile_hook = lambda h: None
    sys.modules["antenv.axon_hooks"] = mod
    # avoid S3 artifact upload attempts during local profile processing
    try:
        from concourse import bass_utils as _bu
        _bu.upload_artifacts = lambda tmpdir: tmpdir
    except Exception:
        pass


def kernel(x, W_qkv, b_qkv, W_out, b_out, _trace=False):
    x = np.asarray(x, dtype=np.float32)
    nc = _get_nc()
    shared = _prep_shared(W_qkv, b_qkv, W_out, b_out)
    masks = [_masks_for(0), _masks_for(1)]

    in_maps = []
    for core in range(8):
        b, h = divmod(core, 2)
        lo = h * HALF - WINDOW
        hi = lo + ROWS
        xh = np.zeros((ROWS, E), dtype=np.float32)
        s0, s1 = max(lo, 0), min(hi, S)
        xh[s0 - lo:s1 - lo] = x[b, s0:s1]
        in_maps.append({
            "xT": np.ascontiguousarray(xh.T).astype(BF16_NP),
            "masks": masks[h],
            **shared,
        })

    kwargs = {}
    if _trace:
        _install_ntff_shim()
        kwargs = dict(trace=True, trace_cores=[0])
    res = run_bass_kernel_spmd(nc, in_maps, core_ids=list(range(8)), **kwargs)

    out = np.empty((B, S, E), dtype=np.float32)
    for core in range(8):
        b, h = divmod(core, 2)
        out[b, h * HALF:(h + 1) * HALF] = res.results[core]["out"].astype(np.float32)
    if _trace:
        return out, res
    return out


# revision 9
# speedup vs baseline: 1.5585x; 1.2024x over previous
"""LocalWindowAttention Trainium2 kernel (Bass/Tile), 8-core SPMD.

Problem: x[B=4, S=4096, E=512] -> out[B, S, E]
  qkv = x @ W_qkv + b_qkv ; q,k,v = split(qkv)
  scores = (q @ k.T) / sqrt(E), banded mask |i-j| <= 64, softmax
  out = (attn @ v) @ W_out + b_out

Sharding: 8 cores = (batch b in 0..3) x (seq half h in 0..1). Each core owns
2048 query rows and loads a 64-row halo of x on each side (zero-padded at
sequence boundaries), computing q/k/v locally - no collectives.

Key structural choices:
  - W_out is folded into the v-projection on the host:
      (attn @ v) @ W_out = attn @ (x @ (W_v @ W_out))
    so the output projection disappears from the kernel. Since attention
    rows sum to 1, the output bias (b_v @ W_out + b_out) is folded into
    the v rows themselves (v'' = v' + b_vo added during the PSUM->SBUF
    copy), which makes softmax normalization a pure per-partition scale.
  - All matmul operands are bf16 (1 cycle/row at any moving size, FWL
    weight loads, half the DMA bytes). PSUM accumulation stays fp32.
  - Scores are computed TRANSPOSED, [key, query], with k-chunks as the
    stationary operand: the exp output is directly the stationary operand
    of the attended matmul -> no PE transposes at all.
  - The band mask is MULTIPLICATIVE (0/1 bf16) applied after exp on the
    gpsimd engine (SBUF-only op; raw scores are O(1) so unmasked exp is
    safe), keeping both the DVE and the PSUM out of that step.
  - Row sums for softmax come from a ones-column appended to the v tiles
    (attended matmul emits [q, 256 feats + rowsum] per half); the final
    normalize is scalar-engine activation with per-partition scale 1/rowsum.
  - Inputs stream on two HW DMA queues (SP: xT + output, ACT: weights),
    ordered so the PE starts in ~5us and never starves.
"""

import sys

sys.path.insert(0, "/opt/trn_rl_repo")

import numpy as np
import ml_dtypes

import concourse.bass as bass  # noqa: F401  (registers types)
import concourse.tile as tile
from concourse import bacc, mybir
from concourse.bass_utils import run_bass_kernel_spmd

F32 = mybir.dt.float32
BF16 = mybir.dt.bfloat16
BF16_NP = ml_dtypes.bfloat16

B, S, E = 4, 4096, 512
WINDOW = 64
HALF = S // 2              # 2048 query rows per core
ROWS = HALF + 2 * WINDOW   # 2176 local rows incl. halo
NT = HALF // 128           # 16 query subtiles per core
NCH = ROWS // 128          # 17 v chunks

# xT column DMA slices (small first slice -> earliest possible first matmul)
DSLC = [(0, 256), (256, 512), (768, 512), (1280, 512), (1792, 384)]
# qT matmul groups in xT col space (queries live at local rows [64, 2112))
QSLC = [(64, 192), (256, 512), (768, 512), (1280, 512), (1792, 320)]
# kT matmul groups (full local rows)
KSLC = [(0, 256), (256, 512), (768, 512), (1280, 512), (1792, 384)]

_NC_CACHE = {}


def _build():
    nc = bacc.Bacc("TRN2", target_bir_lowering=False, debug=False, num_devices=8)

    xT_d = nc.dram_tensor("xT", [E, ROWS], BF16, kind="ExternalInput")
    wqkv_d = nc.dram_tensor("wqkv", [E, 3 * E], BF16, kind="ExternalInput")
    bqk_d = nc.dram_tensor("bqk", [128, 8], F32, kind="ExternalInput")
    mask_d = nc.dram_tensor("masks", [128, 768], BF16, kind="ExternalInput")
    brep_d = nc.dram_tensor("brep", [128, E], F32, kind="ExternalInput")
    out_d = nc.dram_tensor("out", [HALF, E], BF16, kind="ExternalOutput")

    ACT = mybir.ActivationFunctionType
    ALU = mybir.AluOpType

    with tile.TileContext(nc) as tc:
        with (
            tc.tile_pool(name="const", bufs=1) as const,
            tc.tile_pool(name="big", bufs=1) as big,
        ):
            # ---- constants ----
            wq_sb = [const.tile([128, 3 * E], BF16, name=f"wq{e}", tag=f"wq{e}")
                     for e in range(4)]
            bqk_sb = const.tile([128, 8], F32, name="bqk", tag="bqk")
            mask_sb = const.tile([128, 6, 128], BF16, name="msk", tag="msk")
            brep_sb = const.tile([128, 2, 256], F32, name="brep", tag="brep")

            # ---- persistent products ----
            qT = [big.tile([128, HALF], BF16, name=f"qT{f}", tag=f"qT{f}")
                  for f in range(4)]
            kT = [big.tile([128, ROWS], BF16, name=f"kT{f}", tag=f"kT{f}")
                  for f in range(4)]
            # v rows with W_out and output bias folded in; per 128-row chunk:
            # [h, 257] where col 256 of each half is 1.0 (rowsum column)
            vaug = [big.tile([128, 2, 257], BF16, name=f"v{r}", tag=f"v{r}")
                    for r in range(NCH)]

            xTp = [big.tile([128, ROWS], BF16, name=f"xT{e}", tag=f"xT{e}")
                   for e in range(4)]

            # ones columns for the rowsum trick (off critical path)
            for r in range(NCH):
                nc.gpsimd.memset(vaug[r][:, :, 256:257], 1.0)

            # ---- input DMAs ----
            # ACT queue: biases, then weight chunks interleaved in the order
            # the projection groups consume them. SP queue: xT slice-major.
            nc.scalar.dma_start(out=bqk_sb, in_=bqk_d[:, :])
            for e in range(4):
                nc.sync.dma_start(out=xTp[e][:, 0:256],
                                  in_=xT_d[128 * e:128 * (e + 1), 0:256])
            for e in range(4):
                nc.scalar.dma_start(out=wq_sb[e][:, 0:E],
                                    in_=wqkv_d[128 * e:128 * (e + 1), 0:E])
            for e in range(4):
                nc.scalar.dma_start(out=wq_sb[e][:, E:2 * E],
                                    in_=wqkv_d[128 * e:128 * (e + 1), E:2 * E])
            for (c0, w) in DSLC[1:]:
                for e in range(4):
                    nc.sync.dma_start(out=xTp[e][:, c0:c0 + w],
                                      in_=xT_d[128 * e:128 * (e + 1), c0:c0 + w])
            for e in range(4):
                nc.sync.dma_start(out=wq_sb[e][:, 2 * E:3 * E],
                                  in_=wqkv_d[128 * e:128 * (e + 1), 2 * E:3 * E])
            nc.sync.dma_start(out=mask_sb[:, :, :], in_=mask_d[:, :])
            nc.sync.dma_start(out=brep_sb[:, :, :], in_=brep_d[:, :])

            # ---- phase 1: projections ----
            with tc.tile_pool(name="pp", bufs=4, space="PSUM") as pp:
                # qT / kT slice-major so compute starts after the first slice
                for si in range(5):
                    q0, qn = QSLC[si]
                    for f in range(4):
                        ps = pp.tile([128, 512], F32,
                                     name=f"pq{f}_{si}", tag="pp")
                        for e in range(4):
                            nc.tensor.matmul(
                                ps[:, :qn],
                                wq_sb[e][:, 128 * f:128 * (f + 1)],
                                xTp[e][:, q0:q0 + qn],
                                start=(e == 0), stop=(e == 3),
                            )
                        nc.vector.tensor_scalar_add(
                            qT[f][:, q0 - 64:q0 - 64 + qn], ps[:, :qn],
                            bqk_sb[:, f:f + 1],
                        )
                    k0, kn = KSLC[si]
                    for f in range(4):
                        ps = pp.tile([128, 512], F32,
                                     name=f"pk{f}_{si}", tag="pp")
                        for e in range(4):
                            nc.tensor.matmul(
                                ps[:, :kn],
                                wq_sb[e][:, E + 128 * f:E + 128 * (f + 1)],
                                xTp[e][:, k0:k0 + kn],
                                start=(e == 0), stop=(e == 3),
                            )
                        nc.scalar.activation(
                            out=kT[f][:, k0:k0 + kn], in_=ps[:, :kn],
                            func=ACT.Identity, bias=bqk_sb[:, 4 + f:5 + f],
                        )
                # v'' = x @ (W_v @ W_out) + (b_v @ W_out + b_out), natural
                # [rows, feat] layout (bias add fused into the PSUM copy)
                for r in range(NCH):
                    ps = pp.tile([128, 2, 256], F32, name=f"pv{r}", tag="pp")
                    for e in range(4):
                        nc.tensor.matmul(
                            ps[:, :, :],
                            xTp[e][:, 128 * r:128 * (r + 1)],
                            wq_sb[e][:, 2 * E:3 * E],
                            start=(e == 0), stop=(e == 3),
                        )
                    nc.vector.tensor_add(
                        vaug[r][:, :, 0:256], ps[:, :, :], brep_sb[:, :, :])

            # ---- phase 2: banded attention, output written directly ----
            with (
                tc.tile_pool(name="attn", bufs=3) as attn,
                tc.tile_pool(name="ps_s", bufs=3, space="PSUM") as ps_s,
                tc.tile_pool(name="ps_a", bufs=4, space="PSUM") as ps_a,
            ):
                for t in range(NT):
                    # scores transposed: [key, query], two 128-key chunks
                    ps3 = ps_s.tile([128, 2, 128], F32, name=f"s{t}", tag="ps_s")
                    for kc in range(2):
                        for f in range(4):
                            nc.tensor.matmul(
                                ps3[:, kc:kc + 1, :],
                                kT[f][:, 128 * (t + kc):128 * (t + kc + 1)],
                                qT[f][:, 128 * t:128 * (t + 1)],
                                start=(f == 0), stop=(f == 3),
                            )
                    # exp (raw scores are O(1)), then multiplicative 0/1
                    # band mask on gpsimd (SBUF-only), per key-chunk
                    mi = 0 if t == 0 else (2 if t == NT - 1 else 1)
                    ept = attn.tile([128, 2, 128], BF16, name=f"pe{t}", tag="pe")
                    nc.scalar.activation(
                        out=ept[:, :, :], in_=ps3[:, :, :], func=ACT.Exp)
                    for kc in range(2):
                        nc.gpsimd.tensor_tensor(
                            ept[:, kc:kc + 1, :], ept[:, kc:kc + 1, :],
                            mask_sb[:, 2 * mi + kc:2 * mi + kc + 1, :],
                            op=ALU.mult)
                    # attended (unnormalized) + rowsum via the ones column
                    paA = ps_a.tile([128, 257], F32, name=f"paA{t}", tag="ps_a")
                    paB = ps_a.tile([128, 257], F32, name=f"paB{t}", tag="ps_a")
                    for kc in range(2):
                        nc.tensor.matmul(
                            paA[:, :], ept[:, kc:kc + 1, :],
                            vaug[t + kc][:, 0:1, :],
                            start=(kc == 0), stop=(kc == 1),
                        )
                        nc.tensor.matmul(
                            paB[:, :], ept[:, kc:kc + 1, :],
                            vaug[t + kc][:, 1:2, :],
                            start=(kc == 0), stop=(kc == 1),
                        )
                    rd = attn.tile([128, 1], F32, name=f"rd{t}", tag="rd")
                    nc.vector.reciprocal(rd[:], paA[:, 256:257])
                    # out = attended * (1/rowsum); bias already in v rows
                    ost = attn.tile([128, 512], BF16, name=f"ost{t}", tag="ost")
                    nc.scalar.activation(
                        out=ost[:, 0:256], in_=paA[:, 0:256],
                        func=ACT.Identity, scale=rd[:])
                    nc.vector.tensor_scalar_mul(
                        ost[:, 256:512], paB[:, 0:256], rd[:])
                    nc.sync.dma_start(
                        out=out_d[128 * t:128 * (t + 1), :], in_=ost[:])
    nc.compile()
    return nc


def _get_nc():
    if "nc" not in _NC_CACHE:
        _NC_CACHE["nc"] = _build()
    return _NC_CACHE["nc"]


def _prep_shared(W_qkv, b_qkv, W_out, b_out):
    scale = 1.0 / np.sqrt(np.float64(E))
    W = np.array(W_qkv, dtype=np.float64)
    Wo = np.array(W_out, dtype=np.float64)
    b = np.array(b_qkv, dtype=np.float64)
    bo = np.array(b_out, dtype=np.float64)

    wq = W[:, :E] * scale
    wk = W[:, E:2 * E]
    wvo = W[:, 2 * E:3 * E] @ Wo          # fold output projection into v
    wqkv = np.concatenate([wq, wk, wvo], axis=1)

    bq = b[:E] * scale
    bk = b[E:2 * E]
    bqk = np.stack([*(bq.reshape(4, 128)), *(bk.reshape(4, 128))], axis=1)
    bvo = b[2 * E:3 * E] @ Wo + bo        # folded output bias

    shared = {
        "wqkv": np.ascontiguousarray(wqkv.astype(np.float32)).astype(BF16_NP),
        "bqk": np.ascontiguousarray(bqk.astype(np.float32)),
        "brep": np.ascontiguousarray(
            np.tile(bvo.astype(np.float32)[None, :], (128, 1))),
    }
    return shared


def _masks_for(h: int) -> np.ndarray:
    """Multiplicative 0/1 masks in TRANSPOSED [key-in-chunk, kc, query]
    layout. Variant blocks along dim1: [t0 (2x128) | interior | t_last]."""
    j = np.arange(128)[:, None, None]     # key index within chunk
    kc = np.arange(2)[None, :, None]
    i = np.arange(128)[None, None, :]     # query index within tile
    jj = 128 * kc + j                     # key position in the 256 window
    band = (jj - i >= 0) & (jj - i <= 2 * WINDOW)
    m_mid = band
    m_t0 = band & (jj >= 64) if h == 0 else band
    m_tl = band & (jj < 192) if h == 1 else band
    stacked = np.concatenate([m_t0, m_mid, m_tl], axis=1)   # [128, 6, 128]
    return np.ascontiguousarray(
        stacked.reshape(128, 768).astype(np.float32)).astype(BF16_NP)


def _install_ntff_shim():
    """The agent image's antenv lacks axon_hooks; synthesize it from the
    boot module's ctypes NTFF driver so trace=True can capture HW timing."""
    import types
    if "antenv.axon_hooks" in sys.modules:
        return
    try:
        from trn_agent_boot.trn_boot import _ntff_profile_via_ctypes
        hook = _ntff_profile_via_ctypes("/opt/axon/libaxon_pjrt.so")
    except Exception:
        hook = None
    mod = types.ModuleType("antenv.axon_hooks")
    mod.get_axon_ntff_profile_hook = lambda: hook
    mod.set_axon_ntff_profile_hook = lambda h: None
    sys.modules["antenv.axon_hooks"] = mod
    # avoid S3 artifact upload attempts during local profile processing
    try:
        from concourse import bass_utils as _bu
        _bu.upload_artifacts = lambda tmpdir: tmpdir
    except Exception:
        pass


def kernel(x, W_qkv, b_qkv, W_out, b_out, _trace=False):
    x = np.asarray(x, dtype=np.float32)
    nc = _get_nc()
    shared = _prep_shared(W_qkv, b_qkv, W_out, b_out)
    masks = [_masks_for(0), _masks_for(1)]

    in_maps = []
    for core in range(8):
        b, h = divmod(core, 2)
        lo = h * HALF - WINDOW
        hi = lo + ROWS
        xh = np.zeros((ROWS, E), dtype=np.float32)
        s0, s1 = max(lo, 0), min(hi, S)
        xh[s0 - lo:s1 - lo] = x[b, s0:s1]
        in_maps.append({
            "xT": np.ascontiguousarray(xh.T).astype(BF16_NP),
            "masks": masks[h],
            **shared,
        })

    kwargs = {}
    if _trace:
        _install_ntff_shim()
        kwargs = dict(trace=True, trace_cores=[0])
    res = run_bass_kernel_spmd(nc, in_maps, core_ids=list(range(8)), **kwargs)

    out = np.empty((B, S, E), dtype=np.float32)
    for core in range(8):
        b, h = divmod(core, 2)
        out[b, h * HALF:(h + 1) * HALF] = res.results[core]["out"].astype(np.float32)
    if _trace:
        return out, res
    return out


# revision 11
# speedup vs baseline: 1.6137x; 1.0355x over previous
"""LocalWindowAttention Trainium2 kernel (Bass/Tile), 8-core SPMD.

Problem: x[B=4, S=4096, E=512] -> out[B, S, E]
  qkv = x @ W_qkv + b_qkv ; q,k,v = split(qkv)
  scores = (q @ k.T) / sqrt(E), banded mask |i-j| <= 64, softmax
  out = (attn @ v) @ W_out + b_out

Sharding: 8 cores = (batch b in 0..3) x (seq half h in 0..1). Each core owns
2048 query rows and loads a 64-row halo of x on each side (zero-padded at
sequence boundaries), computing q/k/v locally - no collectives.

Key structural choices:
  - W_out is folded into the v-projection on the host:
      (attn @ v) @ W_out = attn @ (x @ (W_v @ W_out))
    so the output projection disappears from the kernel. Since attention
    rows sum to 1, the output bias (b_v @ W_out + b_out) is folded into
    the v rows themselves (v'' = v' + b_vo added during the PSUM->SBUF
    copy), which makes softmax normalization a pure per-partition scale.
  - All matmul operands are bf16 (1 cycle/row at any moving size, FWL
    weight loads, half the DMA bytes). PSUM accumulation stays fp32.
  - Scores are computed TRANSPOSED, [key, query], with k-chunks as the
    stationary operand: the exp output is directly the stationary operand
    of the attended matmul -> no PE transposes at all.
  - The band mask is MULTIPLICATIVE (0/1 bf16) applied after exp on the
    gpsimd engine (SBUF-only op; raw scores are O(1) so unmasked exp is
    safe), keeping both the DVE and the PSUM out of that step.
  - Row sums for softmax come from a ones-column appended to the v tiles
    (attended matmul emits [q, 256 feats + rowsum] per half); the final
    normalize is scalar-engine activation with per-partition scale 1/rowsum.
  - Inputs stream on two HW DMA queues (SP: xT + output, ACT: weights),
    ordered so the PE starts in ~5us and never starves.
"""

import sys

sys.path.insert(0, "/opt/trn_rl_repo")

import numpy as np
import ml_dtypes

import concourse.bass as bass  # noqa: F401  (registers types)
import concourse.tile as tile
from concourse import bacc, mybir
from concourse.bass_utils import run_bass_kernel_spmd

F32 = mybir.dt.float32
BF16 = mybir.dt.bfloat16
BF16_NP = ml_dtypes.bfloat16

B, S, E = 4, 4096, 512
WINDOW = 64
HALF = S // 2              # 2048 query rows per core
ROWS = HALF + 2 * WINDOW   # 2176 local rows incl. halo
NT = HALF // 128           # 16 query subtiles per core
NCH = ROWS // 128          # 17 v chunks

# xT column DMA slices (small first slice -> earliest possible first matmul)
DSLC = [(0, 256), (256, 512), (768, 512), (1280, 512), (1792, 384)]
# qT matmul groups in xT col space (queries live at local rows [64, 2112))
QSLC = [(64, 192), (256, 512), (768, 512), (1280, 512), (1792, 320)]
# kT matmul groups (full local rows)
KSLC = [(0, 256), (256, 512), (768, 512), (1280, 512), (1792, 384)]

_NC_CACHE = {}


def _build():
    nc = bacc.Bacc("TRN2", target_bir_lowering=False, debug=False, num_devices=8)

    xT_d = nc.dram_tensor("xT", [E, ROWS], BF16, kind="ExternalInput")
    wqkv_d = nc.dram_tensor("wqkv", [E, 3 * E], BF16, kind="ExternalInput")
    bqk_d = nc.dram_tensor("bqk", [128, 8], F32, kind="ExternalInput")
    mask_d = nc.dram_tensor("masks", [128, 768], BF16, kind="ExternalInput")
    brep_d = nc.dram_tensor("brep", [128, E], F32, kind="ExternalInput")
    out_d = nc.dram_tensor("out", [HALF, E], BF16, kind="ExternalOutput")

    ACT = mybir.ActivationFunctionType
    ALU = mybir.AluOpType

    with tile.TileContext(nc) as tc:
        with (
            tc.tile_pool(name="const", bufs=1) as const,
            tc.tile_pool(name="big", bufs=1) as big,
        ):
            # ---- constants ----
            wq_sb = [const.tile([128, 3 * E], BF16, name=f"wq{e}", tag=f"wq{e}")
                     for e in range(4)]
            bqk_sb = const.tile([128, 8], F32, name="bqk", tag="bqk")
            mask_sb = const.tile([128, 6, 128], BF16, name="msk", tag="msk")
            brep_sb = const.tile([128, 2, 256], F32, name="brep", tag="brep")

            # ---- persistent products ----
            qT = [big.tile([128, HALF], BF16, name=f"qT{f}", tag=f"qT{f}")
                  for f in range(4)]
            kT = [big.tile([128, ROWS], BF16, name=f"kT{f}", tag=f"kT{f}")
                  for f in range(4)]
            # v rows with W_out and output bias folded in; per 128-row chunk:
            # [h, 257] where col 256 of each half is 1.0 (rowsum column)
            vaug = [big.tile([128, 2, 257], BF16, name=f"v{r}", tag=f"v{r}")
                    for r in range(NCH)]

            xTp = [big.tile([128, ROWS], BF16, name=f"xT{e}", tag=f"xT{e}")
                   for e in range(4)]

            # ones columns for the rowsum trick (off critical path)
            for r in range(NCH):
                nc.gpsimd.memset(vaug[r][:, :, 256:257], 1.0)

            # ---- input DMAs ----
            # ACT queue: biases, then weight chunks interleaved in the order
            # the projection groups consume them. SP queue: xT slice-major.
            nc.scalar.dma_start(out=bqk_sb, in_=bqk_d[:, :])
            for e in range(4):
                nc.sync.dma_start(out=xTp[e][:, 0:256],
                                  in_=xT_d[128 * e:128 * (e + 1), 0:256])
            for e in range(4):
                nc.scalar.dma_start(out=wq_sb[e][:, 0:E],
                                    in_=wqkv_d[128 * e:128 * (e + 1), 0:E])
            for e in range(4):
                nc.scalar.dma_start(out=wq_sb[e][:, E:2 * E],
                                    in_=wqkv_d[128 * e:128 * (e + 1), E:2 * E])
            for (c0, w) in DSLC[1:]:
                for e in range(4):
                    nc.sync.dma_start(out=xTp[e][:, c0:c0 + w],
                                      in_=xT_d[128 * e:128 * (e + 1), c0:c0 + w])
            for e in range(4):
                nc.sync.dma_start(out=wq_sb[e][:, 2 * E:3 * E],
                                  in_=wqkv_d[128 * e:128 * (e + 1), 2 * E:3 * E])
            nc.sync.dma_start(out=mask_sb[:, :, :], in_=mask_d[:, :])
            nc.sync.dma_start(out=brep_sb[:, :, :], in_=brep_d[:, :])

            # ---- phase 1: projections ----
            with tc.tile_pool(name="pp", bufs=4, space="PSUM") as pp:
                # qT / kT slice-major so compute starts after the first slice
                for si in range(5):
                    q0, qn = QSLC[si]
                    for f in range(4):
                        ps = pp.tile([128, 512], F32,
                                     name=f"pq{f}_{si}", tag="pp")
                        for e in range(4):
                            nc.tensor.matmul(
                                ps[:, :qn],
                                wq_sb[e][:, 128 * f:128 * (f + 1)],
                                xTp[e][:, q0:q0 + qn],
                                start=(e == 0), stop=(e == 3),
                            )
                        nc.vector.tensor_scalar_add(
                            qT[f][:, q0 - 64:q0 - 64 + qn], ps[:, :qn],
                            bqk_sb[:, f:f + 1],
                        )
                    k0, kn = KSLC[si]
                    for f in range(4):
                        ps = pp.tile([128, 512], F32,
                                     name=f"pk{f}_{si}", tag="pp")
                        for e in range(4):
                            nc.tensor.matmul(
                                ps[:, :kn],
                                wq_sb[e][:, E + 128 * f:E + 128 * (f + 1)],
                                xTp[e][:, k0:k0 + kn],
                                start=(e == 0), stop=(e == 3),
                            )
                        nc.scalar.activation(
                            out=kT[f][:, k0:k0 + kn], in_=ps[:, :kn],
                            func=ACT.Identity, bias=bqk_sb[:, 4 + f:5 + f],
                        )
                # v'' = x @ (W_v @ W_out) + (b_v @ W_out + b_out), natural
                # [rows, feat] layout (bias add fused into the PSUM copy)
                for r in range(NCH):
                    ps = pp.tile([128, 2, 256], F32, name=f"pv{r}", tag="pp")
                    for e in range(4):
                        nc.tensor.matmul(
                            ps[:, :, :],
                            xTp[e][:, 128 * r:128 * (r + 1)],
                            wq_sb[e][:, 2 * E:3 * E],
                            start=(e == 0), stop=(e == 3),
                        )
                    nc.vector.tensor_add(
                        vaug[r][:, :, 0:256], ps[:, :, :], brep_sb[:, :, :])

            # ---- phase 2: banded attention, output written directly ----
            with (
                tc.tile_pool(name="attn", bufs=4) as attn,
                tc.tile_pool(name="ps_s", bufs=4, space="PSUM") as ps_s,
                tc.tile_pool(name="ps_a", bufs=4, space="PSUM") as ps_a,
            ):
                for t in range(NT):
                    # scores transposed: [key, query], two 128-key chunks
                    ps3 = ps_s.tile([128, 2, 128], F32, name=f"s{t}", tag="ps_s")
                    for kc in range(2):
                        for f in range(4):
                            nc.tensor.matmul(
                                ps3[:, kc:kc + 1, :],
                                kT[f][:, 128 * (t + kc):128 * (t + kc + 1)],
                                qT[f][:, 128 * t:128 * (t + 1)],
                                start=(f == 0), stop=(f == 3),
                            )
                    # exp (raw scores are O(1)), then multiplicative 0/1
                    # band mask on gpsimd (SBUF-only), per key-chunk
                    mi = 0 if t == 0 else (2 if t == NT - 1 else 1)
                    ept = attn.tile([128, 2, 128], BF16, name=f"pe{t}", tag="pe")
                    nc.scalar.activation(
                        out=ept[:, :, :], in_=ps3[:, :, :], func=ACT.Exp)
                    nc.vector.tensor_tensor(
                        ept[:, :, :], ept[:, :, :],
                        mask_sb[:, 2 * mi:2 * mi + 2, :], op=ALU.mult)
                    # attended (unnormalized) + rowsum via the ones column
                    paA = ps_a.tile([128, 257], F32, name=f"paA{t}", tag="ps_a")
                    paB = ps_a.tile([128, 257], F32, name=f"paB{t}", tag="ps_a")
                    for kc in range(2):
                        nc.tensor.matmul(
                            paA[:, :], ept[:, kc:kc + 1, :],
                            vaug[t + kc][:, 0:1, :],
                            start=(kc == 0), stop=(kc == 1),
                        )
                        nc.tensor.matmul(
                            paB[:, :], ept[:, kc:kc + 1, :],
                            vaug[t + kc][:, 1:2, :],
                            start=(kc == 0), stop=(kc == 1),
                        )
                    rd = attn.tile([128, 1], F32, name=f"rd{t}", tag="rd")
                    nc.vector.reciprocal(rd[:], paA[:, 256:257])
                    # out = attended * (1/rowsum); bias already in v rows
                    ost = attn.tile([128, 512], BF16, name=f"ost{t}", tag="ost")
                    nc.scalar.activation(
                        out=ost[:, 0:256], in_=paA[:, 0:256],
                        func=ACT.Identity, scale=rd[:])
                    nc.vector.tensor_scalar_mul(
                        ost[:, 256:512], paB[:, 0:256], rd[:])
                    nc.sync.dma_start(
                        out=out_d[128 * t:128 * (t + 1), :], in_=ost[:])
    nc.compile()
    return nc


def _get_nc():
    if "nc" not in _NC_CACHE:
        _NC_CACHE["nc"] = _build()
    return _NC_CACHE["nc"]


def _prep_shared(W_qkv, b_qkv, W_out, b_out):
    scale = 1.0 / np.sqrt(np.float64(E))
    W = np.array(W_qkv, dtype=np.float64)
    Wo = np.array(W_out, dtype=np.float64)
    b = np.array(b_qkv, dtype=np.float64)
    bo = np.array(b_out, dtype=np.float64)

    wq = W[:, :E] * scale
    wk = W[:, E:2 * E]
    wvo = W[:, 2 * E:3 * E] @ Wo          # fold output projection into v
    wqkv = np.concatenate([wq, wk, wvo], axis=1)

    bq = b[:E] * scale
    bk = b[E:2 * E]
    bqk = np.stack([*(bq.reshape(4, 128)), *(bk.reshape(4, 128))], axis=1)
    bvo = b[2 * E:3 * E] @ Wo + bo        # folded output bias

    shared = {
        "wqkv": np.ascontiguousarray(wqkv.astype(np.float32)).astype(BF16_NP),
        "bqk": np.ascontiguousarray(bqk.astype(np.float32)),
        "brep": np.ascontiguousarray(
            np.tile(bvo.astype(np.float32)[None, :], (128, 1))),
    }
    return shared


def _masks_for(h: int) -> np.ndarray:
    """Multiplicative 0/1 masks in TRANSPOSED [key-in-chunk, kc, query]
    layout. Variant blocks along dim1: [t0 (2x128) | interior | t_last]."""
    j = np.arange(128)[:, None, None]     # key index within chunk
    kc = np.arange(2)[None, :, None]
    i = np.arange(128)[None, None, :]     # query index within tile
    jj = 128 * kc + j                     # key position in the 256 window
    band = (jj - i >= 0) & (jj - i <= 2 * WINDOW)
    m_mid = band
    m_t0 = band & (jj >= 64) if h == 0 else band
    m_tl = band & (jj < 192) if h == 1 else band
    stacked = np.concatenate([m_t0, m_mid, m_tl], axis=1)   # [128, 6, 128]
    return np.ascontiguousarray(
        stacked.reshape(128, 768).astype(np.float32)).astype(BF16_NP)


def _install_ntff_shim():
    """The agent image's antenv lacks axon_hooks; synthesize it from the
    boot module's ctypes NTFF driver so trace=True can capture HW timing."""
    import types
    if "antenv.axon_hooks" in sys.modules:
        return
    try:
        from trn_agent_boot.trn_boot import _ntff_profile_via_ctypes
        hook = _ntff_profile_via_ctypes("/opt/axon/libaxon_pjrt.so")
    except Exception:
        hook = None
    mod = types.ModuleType("antenv.axon_hooks")
    mod.get_axon_ntff_profile_hook = lambda: hook
    mod.set_axon_ntff_profile_hook = lambda h: None
    sys.modules["antenv.axon_hooks"] = mod
    # avoid S3 artifact upload attempts during local profile processing
    try:
        from concourse import bass_utils as _bu
        _bu.upload_artifacts = lambda tmpdir: tmpdir
    except Exception:
        pass


def kernel(x, W_qkv, b_qkv, W_out, b_out, _trace=False):
    x = np.asarray(x, dtype=np.float32)
    nc = _get_nc()
    shared = _prep_shared(W_qkv, b_qkv, W_out, b_out)
    masks = [_masks_for(0), _masks_for(1)]

    in_maps = []
    for core in range(8):
        b, h = divmod(core, 2)
        lo = h * HALF - WINDOW
        hi = lo + ROWS
        xh = np.zeros((ROWS, E), dtype=np.float32)
        s0, s1 = max(lo, 0), min(hi, S)
        xh[s0 - lo:s1 - lo] = x[b, s0:s1]
        in_maps.append({
            "xT": np.ascontiguousarray(xh.T).astype(BF16_NP),
            "masks": masks[h],
            **shared,
        })

    kwargs = {}
    if _trace:
        _install_ntff_shim()
        kwargs = dict(trace=True, trace_cores=[0])
    res = run_bass_kernel_spmd(nc, in_maps, core_ids=list(range(8)), **kwargs)

    out = np.empty((B, S, E), dtype=np.float32)
    for core in range(8):
        b, h = divmod(core, 2)
        out[b, h * HALF:(h + 1) * HALF] = res.results[core]["out"].astype(np.float32)
    if _trace:
        return out, res
    return out
